# revision 15
# baseline (speedup 1.0000x reference)
"""LoRA linear y = x @ (B@A).T computed low-rank: y = (x @ A.T) @ B.T.

Sharding: data-parallel over tokens (B*S = 16384) across 8 NeuronCores,
2048 tokens/core; lora_A / lora_B replicated (tiny). No collectives.

bf16 end-to-end (rel-err gate 2e-2; this path lands ~3.5e-3). Host casts
x to bf16 and pre-transposes per core to xT (d-major), so the kernel
needs zero on-chip transposes. HBM traffic/core: 16 MB in + 16 MB out
-> DMA floor ~85us at the measured ~26 GB/s x 16 SDMA engines.

Pipeline (NSPLIT token splits/core, default 4 x 512 tokens):
 - mm1(s): tT_s[16, TPS] += A.T-chunk.T @ xT-slab over 32 d-chunks.
   2 MB slabs (16 chunks) are loaded ONE WINDOW AHEAD on the sync ring:
   big DMAs keep the 16 SDMA engines packed, and resident slabs mean
   the PE never stalls on a split boundary.
 - Window order: mm1(s) first, then tcopy(s) (so it sits ahead of the
   window's PSUM drains in the DVE/ACT FIFOs -- a late tcopy convoys
   behind them, stalls the PE ~5us per boundary, and flips the HAM
   clock-gate cold), then mm2(s-1)'s h-loop.
 - tT -> rows 0-15 of t_pad (bf16, rows 16-127 zeroed once): mm2 runs
   with K zero-padded 16->128 so the PE HAM clock-gate sees full-row
   activity and stays at 8/8 (2.4 GHz; K=16 matmuls throttle to 1.2).
   Warm padded mm2 streams at fill-rate 216ns/MM.
 - mm2: y[128,512] = t_pad_h.T @ bt_pad_nb into single-bank PSUM tiles
   (6-deep rotation; 2-bank tiles serialize back-to-back MMs), drains
   alternate DVE/ACT into paired-h y tiles [128, 2*4096]; 2 MB stores
   alternate sync/scalar rings. ys layout [8, 128, 2, 4096] is
   reassembled on host.
"""

import os
import numpy as np
import ml_dtypes

import concourse.bass as bass
import concourse.mybir as mybir
from concourse.tile import TileContext
from concourse.bass_utils import run_bass_kernel_spmd

N_CORES = 8
B, S, D_IN, D_OUT, R = 4, 4096, 4096, 4096, 16
TOK = B * S
TPC = TOK // N_CORES        # tokens per core: 2048
NC_D = D_IN // 128          # 32 d-chunks
CPS = 16                    # d-chunks per slab (2 MB slabs at TPS=512)
NGS = NC_D // CPS           # 2 slabs per split
BF16 = mybir.dt.bfloat16
F32 = mybir.dt.float32
NP_BF16 = ml_dtypes.bfloat16

NSPLIT = int(os.environ.get("NSPLIT", "4"))
TPS = TPC // NSPLIT         # tokens per split
NH_S = TPS // 128           # token blocks per split
NHH_S = NH_S // 2           # paired-h store groups per split
NQ = TPS // 512             # 512-col matmul slices per split
TPS_BANKS = (TPS * 4 + 2047) // 2048
XB = int(os.environ.get("XB", str(3 * NGS + 1)))
YB = int(os.environ.get("YB", "3"))
YPB = int(os.environ.get("YPB", str((8 - min(NSPLIT, 2) * TPS_BANKS) // 2)))


def _split_drain_waits(nc):
    """This walrus build rejects instructions carrying >1 sem wait; hoist
    extra waits onto preceding single-wait NoOps on the same engine."""
    f = nc.m.functions[0]

    def fix_bb(bb):
        insts = getattr(bb, "instructions", None)
        if insts:
            new = []
            for inst in insts:
                si = inst.sync_info
                if si is not None and si.on_wait is not None and len(si.on_wait) > 1:
                    waits = list(si.on_wait)
                    for w in waits[:-1]:
                        d = mybir.InstNoOp(
                            name=nc.get_next_instruction_name(), ins=[], outs=[]
                        )
                        d.engine = inst.engine
                        d.sync_info = mybir.SyncInfo(on_wait=[w], on_update=[])
                        new.append(d)
                    si.on_wait = [waits[-1]]
                    inst.sync_info = si
                new.append(inst)
            bb.instructions[:] = new
        for sub in getattr(bb, "blocks", []) or []:
            fix_bb(sub)

    for blk in f.blocks:
        fix_bb(blk)


def _build():
    nc = bass.Bass("TRN2", target_bir_lowering=False, debug=False, num_devices=N_CORES)
    # xs[s, g, p, j*TPS + t] = x[token s*TPS+t, (CPS*g+j)*128 + p]  (bf16)
    xs = nc.declare_dram_parameter("xs", [NSPLIT, NGS, 128, CPS * TPS], BF16, isOutput=False)
    # atp[p, c*R + r] = A[r, c*128 + p]  (bf16)
    atp = nc.declare_dram_parameter("atp", [128, NC_D * R], BF16, isOutput=False)
    bt = nc.declare_dram_parameter("bt", [R, D_OUT], BF16, isOutput=False)
    # ys[hh, p, j, o] = y[token (2*hh+j)*128 + p, o]  (host reassembles)
    ys = nc.declare_dram_parameter("ys", [TPC // 256, 128, 2, D_OUT], BF16, isOutput=True)

    with TileContext(nc) as tc:
        with (
            tc.tile_pool(name="const", bufs=1) as cpool,
            tc.tile_pool(name="x", bufs=XB) as xpool,
            tc.tile_pool(name="y", bufs=YB) as ypool,
            tc.tile_pool(name="t_ps", bufs=min(NSPLIT, 2), space="PSUM") as tpsum,
            tc.tile_pool(name="y_ps", bufs=YPB, space="PSUM") as ypsum,
        ):
            at_sb = cpool.tile([128, NC_D * R], BF16)
            nc.scalar.dma_start(out=at_sb[:], in_=atp[:])
            # bt_pad rows 16-127 zeroed on gpsimd (idle engine), rows 0-15 DMA'd
            bt_sb = cpool.tile([128, D_OUT], BF16)
            nc.gpsimd.memset(bt_sb[:], 0.0)
            nc.scalar.dma_start(out=bt_sb[0:R, :], in_=bt[:])
            # t_pad rows 16-127 stay zero for the whole kernel
            t_pad = cpool.tile([128, TPC], BF16)
            nc.vector.memset(t_pad[:], 0.0)

            tps_tiles = [None] * NSPLIT
            slabs = {}

            def emit_loads(s):
                for g in range(NGS):
                    slab = xpool.tile([128, CPS * TPS], BF16, name=f"slab{s}_{g}", tag="slab")
                    nc.sync.dma_start(out=slab[:], in_=xs[s, g])
                    slabs[(s, g)] = slab

            def emit_mm1_slab(s, g):
                if g == 0:
                    tps_tiles[s] = tpsum.tile([R, TPS], F32, name=f"tps{s}", tag="tps")
                tps = tps_tiles[s]
                slab = slabs.pop((s, g))
                for j in range(CPS):
                    c = CPS * g + j
                    for q in range(NQ):
                        nc.tensor.matmul(
                            tps[:, q * 512 : (q + 1) * 512],
                            at_sb[:, c * R : (c + 1) * R],
                            slab[:, j * TPS + q * 512 : j * TPS + (q + 1) * 512],
                            start=(c == 0),
                            stop=(c == NC_D - 1),
                        )

            def emit_tcopy(s):
                tps = tps_tiles[s]
                half = TPS // 2
                nc.vector.tensor_copy(
                    out=t_pad[0:R, s * TPS : s * TPS + half], in_=tps[:, 0:half]
                )
                nc.scalar.activation(
                    out=t_pad[0:R, s * TPS + half : (s + 1) * TPS],
                    in_=tps[:, half:TPS],
                    func=mybir.ActivationFunctionType.Identity,
                )

            def emit_mm2_hh(s, hh):
                ghh = (s * NH_S) // 2 + hh  # global paired-h index
                y_sb = ypool.tile([128, 2 * D_OUT], BF16, name=f"ysb{s}_{hh}", tag="ysb")
                for j in range(2):
                    row = s * TPS + (2 * hh + j) * 128
                    # 8 MMs as 2 rounds over two 2-bank tiles (U,V), MM
                    # order U0 V0 U1 V1 so back-to-back MMs never target
                    # the same bank pair (that serializes fill+drain),
                    # then one [128,1024] drain per tile (DVE / ACT).
                    for r2 in range(2):
                        u = ypsum.tile([128, 1024], F32, name=f"ypu{s}_{hh}_{j}_{r2}", tag="yps")
                        v = ypsum.tile([128, 1024], F32, name=f"ypv{s}_{hh}_{j}_{r2}", tag="yps")
                        # U <- nb 4r2+0, 4r2+1; V <- nb 4r2+2, 4r2+3
                        for half in range(2):
                            for tile, nb in ((u, 4 * r2 + half), (v, 4 * r2 + 2 + half)):
                                nc.tensor.matmul(
                                    tile[:, half * 512 : (half + 1) * 512],
                                    t_pad[:, row : row + 128],
                                    bt_sb[:, nb * 512 : (nb + 1) * 512],
                                    start=True,
                                    stop=True,
                                )
                        base = j * D_OUT + 4 * r2 * 512
                        nc.vector.tensor_copy(
                            out=y_sb[:, base : base + 1024], in_=u[:]
                        )
                        nc.scalar.activation(
                            out=y_sb[:, base + 1024 : base + 2048],
                            in_=v[:],
                            func=mybir.ActivationFunctionType.Identity,
                        )
                nc.scalar.dma_start(out=ys[ghh], in_=y_sb[:])

            # prologue: loads for splits 0-2 (slabs land TWO windows ahead
            # of their mm1, so a window's loads finishing at its end can
            # never stall the next window's mm1 -> tcopy -> mm2 chain),
            # mm1(0) chasing loads(0)
            for s in range(min(3, NSPLIT)):
                emit_loads(s)
            for g in range(NGS):
                emit_mm1_slab(0, g)
            emit_tcopy(0)
            # steady-state window for split s: prefetch loads(s+2); mm1(s)
            # (slabs resident) + early tcopy(s); then mm2(s-1)
            for s in range(1, NSPLIT):
                if s + 2 < NSPLIT:
                    emit_loads(s + 2)
                for g in range(NGS):
                    emit_mm1_slab(s, g)
                emit_tcopy(s)
                for hh in range(NHH_S):
                    emit_mm2_hh(s - 1, hh)
            for hh in range(NHH_S):
                emit_mm2_hh(NSPLIT - 1, hh)

    _split_drain_waits(nc)
    return nc


_NC = None


def _get_nc():
    global _NC
    if _NC is None:
        _NC = _build()
    return _NC


def _prep_inputs(x, lora_A, lora_B):
    x2d = np.asarray(x, dtype=np.float32).reshape(TOK, D_IN)
    A = np.asarray(lora_A, dtype=np.float32)
    Bm = np.asarray(lora_B, dtype=np.float32)
    atp = np.ascontiguousarray(
        A.T.reshape(NC_D, 128, R).transpose(1, 0, 2).reshape(128, NC_D * R)
    ).astype(NP_BF16)
    btv = np.ascontiguousarray(Bm.T).astype(NP_BF16)
    xs_list = []
    for i in range(N_CORES):
        shard = x2d[i * TPC : (i + 1) * TPC].astype(NP_BF16)  # [2048, 4096]
        # [s, t, g, j, p] -> [s, g, p, j, t]
        v = shard.reshape(NSPLIT, TPS, NGS, CPS, 128).transpose(0, 2, 4, 3, 1)
        xs_list.append(
            np.ascontiguousarray(v).reshape(NSPLIT, NGS, 128, CPS * TPS)
        )
    return xs_list, atp, btv


def kernel(x, lora_A, lora_B, _trace=False, _trace_kwargs=None):
    nc = _get_nc()
    xs_list, atp, btv = _prep_inputs(x, lora_A, lora_B)
    in_maps = [
        {"xs": xs_list[i], "atp": atp, "bt": btv} for i in range(N_CORES)
    ]
    res = run_bass_kernel_spmd(
        nc, in_maps, list(range(N_CORES)), trace=_trace, **(_trace_kwargs or {})
    )
    # ys[hh, p, j, o] -> tokens (2*hh+j)*128 + p
    y = np.concatenate(
        [
            np.asarray(res.results[i]["ys"]).transpose(0, 2, 1, 3).reshape(TPC, D_OUT)
            for i in range(N_CORES)
        ],
        axis=0,
    )
    out = y.astype(np.float32).reshape(B, S, D_OUT)
    if _trace:
        return out, res
    return out
